# revision 48
# baseline (speedup 1.0000x reference)
"""Trainium2 Bass kernel for nn_EnsembleNet3 (gnn_message_passing).

Self-contained: takes full inputs (as produced by setup_inputs()), shards the
B=32 graph dim over 8 NeuronCores (4 graphs/core), runs the graph stack fully
on-device. The [B,1536] head is replicated on every core after a single
AllGather of the pooled features (BatchNorm couples graphs); lin weights
stream from HBM in fp16, W-stationary.

Per graph (N=512 nodes):
- kNN-100 for TAGConv: per-row threshold via count-secant iteration on
  Q[i,j] = 2*G[i,j] - n_j (same per-row order as -dist; Q row max is self),
  finished by an exact masked max-8 endgame extracting the 101st and 102nd
  largest; the mask threshold sits mid-gap between them so the fp32 maskT
  recompute (fused K=13 matmul with the threshold row folded into xgT row 12)
  is robust to rounding. Adjacency applied as a dense 0/1 mask matmul on PE
  (float32r) with host-folded hop weights:
  out = h@W~0 + (Mh)@W~1 + (M^2 h)@W~2,  M = mask incl self.
  Node-major M@h obtained by PE-transposing the feature-major aggregate.
- EdgeConv k=3: top-3 indices via max8+max_index on Q; indices marshalled into
  the GPSIMD wrapped-16 layout with PE piece-transposes + a replication
  matmul, gather via indirect_copy. MLPs decomposed as a_i + g_j so only g is
  gathered; max-aggregation commutes with leaky_relu. leaky_relu computed as
  max(x, 0.01x) in one fused DVE op (with free mean-pool accumulation).
"""
import os
from contextlib import ExitStack

import numpy as np

import concourse.bass as bass
import concourse.bacc as bacc
import concourse.tile as tile
from concourse import mybir
from concourse._compat import with_exitstack

F32 = mybir.dt.float32
F32R = mybir.dt.float32r
F16 = mybir.dt.float16
U16 = mybir.dt.uint16
U32 = mybir.dt.uint32
U8 = mybir.dt.uint8
ALU = mybir.AluOpType
ACTF = mybir.ActivationFunctionType
AXX = mybir.AxisListType.X

B, N, F, W = 32, 512, 6, 128
NT = N // 128
GPC = 4
NCORES = 8
K101 = 101
SEL_ITERS = 11
SEL_TARGET = float(K101 + 4)
U_LO, U_HI = -64.0, 64.0
DIM2 = 1536
LIN_D = 5
DVE_COLS = 6     # selection count passes: cols < DVE_COLS on DVE, rest on ACT
LIN_CHUNK = 2    # k-tiles per streamed lin_W chunk (6 chunks per layer)


def _fold_tag(Wk, b):
    W0, W1, W2 = Wk[0], Wk[1], Wk[2]
    c1, c2 = 1.0 / 100.0, 1.0 / 10000.0
    return (
        (W0 - W1 * c1 + W2 * c2).astype(np.float32),
        (W1 * c1 - 2.0 * W2 * c2).astype(np.float32),
        (W2 * c2).astype(np.float32),
        b.astype(np.float32),
    )


def prep_host(inputs, core):
    inp = {k: np.asarray(v) for k, v in inputs.items()}
    x = inp['x'].astype(np.float32).reshape(B, N, F)
    xs = x[GPC * core:GPC * (core + 1)]
    f32 = np.float32

    # --- per-graph input pack [128, 1048]: xgT | xgR | xnm ---
    xt = xs.transpose(0, 2, 1)
    xpack = np.zeros((GPC, 128, 2 * N + 24), f32)
    xpack[:, 0:F, 0:N] = xt
    xpack[:, F:2 * F, 0:N] = 1.0
    xpack[:, 0:F, N:2 * N] = 2.0 * xt
    xpack[:, F:2 * F, N:2 * N] = -(xt * xt)
    xpack[:, 12, N:2 * N] = 1.0
    for t in range(NT):
        xpack[:, :, 2 * N + F * t:2 * N + F * (t + 1)] = xs[:, 128 * t:128 * (t + 1), :]

    # --- const pack [128, cols] ---
    cols = {}
    blobs = []
    off = 0

    def put(name, arr2d):
        nonlocal off
        a = np.asarray(arr2d, f32)
        pad = np.zeros((128, a.shape[1]), f32)
        pad[:a.shape[0]] = a
        cols[name] = (off, a.shape[1])
        blobs.append(pad)
        off += a.shape[1]

    put('eye', np.eye(128, dtype=f32))
    put('iota8', np.broadcast_to(np.arange(8, dtype=f32), (128, 8)))
    rep16 = np.zeros((16, 128), f32)
    for q in range(128):
        rep16[q % 16, q] = 1.0
    put('rep16', rep16)
    # fp32r-rounded weight block: [tagw0|tagw1|tagw2|ec1_a|ec1_g|ec1_w2|
    #                              ec2_a|ec2_g|ec3_a|ec3_g]
    for li, (Wk, bk) in enumerate(
            [(inp['tag1_W'], inp['tag1_b']),
             (inp['tag_W'][0], inp['tag_b'][0]),
             (inp['tag_W'][1], inp['tag_b'][1])]):
        w0, w1, w2, bb = _fold_tag(Wk, bk)
        put(f'tagw{li}', np.concatenate([w0, w1, w2], axis=1))
        put(f'tagb{li}', bb.reshape(128, 1))
    W1 = inp['p1_W1'].astype(f32)
    put('ec1_a', W1[:F] - W1[F:])
    put('ec1_g', W1[F:])
    put('ec1_w2', inp['p1_W2'].astype(f32))
    for f in range(2):
        Wf = inp['pf_W'][f].astype(f32)
        put(f'ec{f+2}_a', Wf[:W] - Wf[W:])
        put(f'ec{f+2}_g', Wf[W:])
    put('ec1_b1', inp['p1_b1'].astype(f32).reshape(128, 1))
    put('ec1_b2', inp['p1_b2'].astype(f32).reshape(128, 1))
    for f in range(2):
        put(f'ec{f+2}_b', inp['pf_b'][f].astype(f32).reshape(128, 1))
    put('bn_scale', inp['bn_gamma'].astype(f32).reshape(12, 128).T)
    put('bn_shift', inp['bn_beta'].astype(f32).reshape(12, 128).T)
    put('outb', np.full((1, 1), float(inp['out_b'][0]), f32))
    put('linB', inp['lin_b'].astype(f32).reshape(LIN_D * 12, 128).T)
    wpack = np.concatenate(blobs, axis=1)
    assert wpack.shape[1] == WPACK_COLS, (wpack.shape, off)
    assert all(cols[k] == WOFF[k] for k in cols), "WOFF mismatch"

    # --- lin weights fp16, W-stationary: [LIN_D, 12 ktile, 128, 1536] ---
    linW = inp['lin_W'].astype(np.float16).reshape(LIN_D, 12, 128, DIM2)
    d = {
        'xpack': np.ascontiguousarray(xpack),
        'wpack': np.ascontiguousarray(wpack),
        'linW': np.ascontiguousarray(linW),
        'outW16': np.ascontiguousarray(
            inp['out_W'].astype(np.float16).reshape(12, 128).T),
    }
    return d


def _woff_table():
    off = 0
    tab = {}

    def put(name, w):
        nonlocal off
        tab[name] = (off, w)
        off += w
    put('eye', 128); put('iota8', 8); put('rep16', 128)
    # fp32r block start
    tab['_r_begin'] = (off, 0)
    for li in range(3):
        put(f'tagw{li}', 384); put(f'tagb{li}', 1)
    tab['_r_end'] = (off, 0)
    put('ec1_a', 128); put('ec1_g', 128); put('ec1_w2', 128)
    for f in range(2):
        put(f'ec{f+2}_a', 128); put(f'ec{f+2}_g', 128)
    put('ec1_b1', 1); put('ec1_b2', 1)
    for f in range(2):
        put(f'ec{f+2}_b', 1)
    put('bn_scale', 12); put('bn_shift', 12)
    put('outb', 1)
    put('linB', LIN_D * 12)
    return tab, off


WOFF, WPACK_COLS = _woff_table()


@with_exitstack
def core_program(ctx: ExitStack, tc: tile.TileContext, io: dict, skip_head=False):
    nc = tc.nc
    P = 128
    SKIP_EC23 = bool(os.environ.get("K_SKIP_EC23"))
    SKIP_EC = bool(os.environ.get("K_SKIP_EC"))
    SKIP_TAG = bool(os.environ.get("K_SKIP_TAG"))

    const = ctx.enter_context(tc.tile_pool(name="const", bufs=1))
    pq = ctx.enter_context(tc.tile_pool(name="pq", bufs=16))
    pmask = ctx.enter_context(tc.tile_pool(name="pmask", bufs=16))
    pwork = ctx.enter_context(tc.tile_pool(name="pwork", bufs=1))
    pbig = ctx.enter_context(tc.tile_pool(name="pbig", bufs=1))
    phT = ctx.enter_context(tc.tile_pool(name="phT", bufs=4))
    pyT = ctx.enter_context(tc.tile_pool(name="pyT", bufs=2))
    phn = ctx.enter_context(tc.tile_pool(name="phn", bufs=2))
    phm = ctx.enter_context(tc.tile_pool(name="phm", bufs=4))
    pq2 = ctx.enter_context(tc.tile_pool(name="pq2", bufs=2))
    pst = ctx.enter_context(tc.tile_pool(name="pst", bufs=1))
    plinw = ctx.enter_context(tc.tile_pool(name="plinw", bufs=2))
    psq = ctx.enter_context(tc.tile_pool(name="psq", bufs=3, space="PSUM"))
    pss = ctx.enter_context(tc.tile_pool(name="pss", bufs=2, space="PSUM"))
    psh = ctx.enter_context(tc.tile_pool(name="psh", bufs=1, space="PSUM"))

    def quad_ps(pp=P, nn=N, dt=F32):
        return psq.tile([pp, nn], dt, tag="quad", name="quad")

    def seq_ps(pp, nn, dt=F32):
        return pss.tile([pp, nn], dt, tag="seq", name="seq")

    def dma(dst, src):
        nc.sync.dma_start(dst, src)

    _cp = [0, False]

    def copy_ps(dst, src):
        if _cp[1] and _cp[0] % 2 == 1:
            nc.vector.tensor_copy(dst, src)
        else:
            nc.scalar.copy(dst, src)
        _cp[0] += 1

    # ---- constants: one packed DMA ----
    wp = const.tile([P, WPACK_COLS], F32, tag="wpack", name="wpack")
    dma(wp[:], io['wpack'][:])

    def wslice(name, rows=128):
        o, w = WOFF[name]
        return wp[0:rows, o:o + w]

    eye = wslice('eye')
    iota8 = wslice('iota8')
    rep16 = wslice('rep16', 16)
    eyer = const.tile([P, P], F32R)
    nc.vector.tensor_copy(eyer[:], eye)
    onesf = const.tile([P, P], F32)
    nc.any.memset(onesf[:], 1.0)

    # fp32r copy of the weight block
    r0 = WOFF['_r_begin'][0]
    r1 = WOFF['_r_end'][0]
    wpr = const.tile([P, r1 - r0], F32R, tag="wpr", name="wpr")
    nc.vector.tensor_copy(wpr[:], wp[:, r0:r1])

    def wslice_r(name, rows=128):
        o, w = WOFF[name]
        return wpr[0:rows, o - r0:o - r0 + w]

    tagw, tagb = [], []
    for li in range(3):
        fin = F if li == 0 else W
        wt = wslice_r(f'tagw{li}', fin)
        tagw.append([wt[:, 128 * k:128 * (k + 1)] for k in range(3)])
        tagb.append(wslice(f'tagb{li}'))

    ec1_a = wslice('ec1_a', F)
    ec1_g = wslice('ec1_g', F)
    ec1_w2 = wslice('ec1_w2')
    ec1_b1 = wslice('ec1_b1')
    ec1_b2 = wslice('ec1_b2')
    ecf_a = [wslice('ec2_a'), wslice('ec3_a')]
    ecf_g = [wslice('ec2_g'), wslice('ec3_g')]
    ecf_b = [wslice('ec2_b'), wslice('ec3_b')]

    # ---- inputs per graph: one packed DMA each ----
    xgT, xgR, xnm = [], [], []
    xps = []
    for g in range(GPC):
        xp = pst.tile([P, 2 * N + 24], F32, tag=f"xpack{g}", name=f"xpack{g}")
        dma(xp[:], io['xpack'][g])
        xps.append(xp)
        xgT.append(xp[:, 0:N])
        xgR.append(xp[:, N:2 * N])
        xnm.append(xp[:, 2 * N:2 * N + 24])

    # fp32r copies of x inputs used in f32r matmuls
    xgT6r, xnmr = [], []
    for g in range(GPC):
        xr = pst.tile([F, N], F32R, tag=f"xgT6r{g}", name=f"xgT6r{g}",
                      padded_shape=[128, N])
        nc.vector.tensor_copy(xr[:], xgT[g][0:F, 0:N])
        xgT6r.append(xr)
        xnr = pst.tile([P, 24], F32R, tag=f"xnmr{g}", name=f"xnmr{g}")
        nc.vector.tensor_copy(xnr[:], xnm[g])
        xnmr.append(xnr)

    # ---- Q = 2G - n_row via augmented matmul (K=12), fp32 exact ----
    Q = [[None] * NT for _ in range(GPC)]
    for g in range(GPC):
        gps = [quad_ps() for _ in range(NT)]
        for t in range(NT):
            nc.tensor.matmul(gps[t][:], xgT[g][0:12, 128 * t:128 * (t + 1)],
                             xgR[g][0:12, 0:N], start=True, stop=True)
        for t in range(NT):
            qt = pq.tile([P, N], F32, tag="Q", name="Q")
            copy_ps(qt[:], gps[t][:])
            Q[g][t] = qt

    # ---- lockstep count-secant selection ----
    NC16 = GPC * NT
    st_u = pst.tile([P, NC16], F32, tag="st_u", name="st_u")
    st_ul = pst.tile([P, NC16], F32, tag="st_ul", name="st_ul")
    st_uh = pst.tile([P, NC16], F32, tag="st_uh", name="st_uh")
    st_cl = pst.tile([P, NC16], F32, tag="st_cl", name="st_cl")
    st_ch = pst.tile([P, NC16], F32, tag="st_ch", name="st_ch")
    cnt = pst.tile([P, NC16], F32, tag="cnt", name="cnt")
    tmp_a = pst.tile([P, NC16], F32, tag="tmp_a", name="tmp_a")
    tmp_b = pst.tile([P, NC16], F32, tag="tmp_b", name="tmp_b")
    tmp_m = pst.tile([P, NC16], U8, tag="tmp_m", name="tmp_m")
    junk_d = pst.tile([P, N], F32, tag="junk_d", name="junk_d")
    junk_a = pst.tile([P, N], F32, tag="junk_a", name="junk_a")
    nc.any.memset(st_ul[:], U_HI)
    nc.any.memset(st_cl[:], 0.0)
    nc.any.memset(st_uh[:], U_LO)
    nc.any.memset(st_ch[:], float(N))
    nc.any.memset(st_u[:], U_HI + (U_LO - U_HI) * (SEL_TARGET / N))

    for it in range(SEL_ITERS):
        for g in range(GPC):
            for t in range(NT):
                col = 4 * g + t
                ucol = st_u[:, col:col + 1]
                ccol = cnt[:, col:col + 1]
                if col < DVE_COLS:
                    nc.vector.tensor_scalar(
                        junk_d[:], Q[g][t][:], ucol, 0.0,
                        op0=ALU.is_ge, op1=ALU.add, accum_out=ccol)
                else:
                    nc.scalar.activation(
                        junk_a[:], Q[g][t][:], ACTF.Sign,
                        bias=ucol, scale=-1.0, accum_out=ccol)
        # ACT cols: c = 256 - s/2
        nc.vector.tensor_scalar(
            cnt[:, DVE_COLS:NC16], cnt[:, DVE_COLS:NC16], -0.5, 256.0,
            op0=ALU.mult, op1=ALU.add)
        nc.vector.tensor_scalar(
            tmp_m[:], cnt[:], float(K101) - 0.5, 0.0, op0=ALU.is_ge)
        nc.vector.copy_predicated(st_uh[:], tmp_m[:], st_u[:])
        nc.vector.copy_predicated(st_ch[:], tmp_m[:], cnt[:])
        nc.vector.tensor_scalar(
            tmp_m[:], cnt[:], float(K101) - 0.5, 0.0, op0=ALU.is_lt)
        nc.vector.copy_predicated(st_ul[:], tmp_m[:], st_u[:])
        nc.vector.copy_predicated(st_cl[:], tmp_m[:], cnt[:])
        if it == SEL_ITERS - 1:
            break
        nc.vector.tensor_tensor(tmp_a[:], st_ch[:], st_cl[:], op=ALU.subtract)
        nc.vector.reciprocal(tmp_a[:], tmp_a[:])
        nc.vector.scalar_tensor_tensor(
            tmp_b[:], st_ch[:], -SEL_TARGET, tmp_a[:], op0=ALU.add, op1=ALU.mult)
        nc.vector.tensor_scalar(
            tmp_b[:], tmp_b[:], 0.05, 0.95, op0=ALU.max, op1=ALU.min)
        nc.vector.tensor_tensor(tmp_a[:], st_ul[:], st_uh[:], op=ALU.subtract)
        nc.vector.tensor_tensor(tmp_a[:], tmp_a[:], tmp_b[:], op=ALU.mult)
        nc.vector.tensor_tensor(st_u[:], st_uh[:], tmp_a[:], op=ALU.add)

    # ---- endgame: exact 101st + 102nd largest of each Q row ----
    # mask threshold sits mid-gap so the maskT fp32 recompute can't flip the
    # boundary neighbor. When pos==0 the 102nd value is below the uh bracket,
    # and uh itself is a valid lower mid-point.
    # acc[col] = -(u101 + u102) via a single two-rank mask (iota in
    # {pos-1, pos}); when pos==0 only u101 lands, patched with uh below.
    acc2 = pst.tile([P, NC16], F32, tag="acc2", name="acc2")
    posh = pst.tile([P, NC16], F32, tag="posh", name="posh")
    nc.vector.tensor_scalar(posh[:], st_ch[:], -float(K101) - 0.5, 0.0,
                            op0=ALU.add)
    for g in range(GPC):
        for t in range(NT):
            col = 4 * g + t
            zm = pwork.tile([P, N], F32, tag="zm", name="zm")
            nc.vector.tensor_scalar(
                zm[:], Q[g][t][:], st_uh[:, col:col + 1], -1e30,
                op0=ALU.is_lt, op1=ALU.mult)
            nc.vector.tensor_tensor(zm[:], zm[:], Q[g][t][:], op=ALU.subtract)
            m8 = pwork.tile([P, 8], F32, tag="m8e", name="m8e")
            nc.vector.max(m8[:], zm[:])
            d8 = pwork.tile([P, 8], F32, tag="d8", name="d8")
            nc.vector.tensor_tensor(
                d8[:], iota8,
                posh[:, col:col + 1].broadcast_to([P, 8]), op=ALU.subtract)
            msk8 = pwork.tile([P, 8], F32, tag="msk8", name="msk8")
            nc.vector.scalar_tensor_tensor(
                msk8[:], d8[:], 1.0, d8[:], op0=ALU.mult, op1=ALU.mult)
            nc.vector.tensor_scalar(msk8[:], msk8[:], 1.0, 0.0, op0=ALU.is_lt)
            j8 = pwork.tile([P, 8], F32, tag="j8", name="j8")
            nc.vector.scalar_tensor_tensor(
                j8[:], m8[:], 1.0, msk8[:], op0=ALU.mult, op1=ALU.mult,
                accum_out=acc2[:, col:col + 1])
    pos0 = pst.tile([P, NC16], U8, tag="pos0", name="pos0")
    nc.vector.tensor_scalar(pos0[:], posh[:], 0.0, 0.0, op0=ALU.is_lt)
    uhadd = pst.tile([P, NC16], F32, tag="uhadd", name="uhadd")
    nc.any.memset(uhadd[:], 0.0)
    nc.vector.copy_predicated(uhadd[:], pos0[:], st_uh[:])
    nc.vector.tensor_scalar(uhadd[:], uhadd[:], 0.5, 0.0, op0=ALU.mult)
    thr = pst.tile([P, NC16], F32, tag="thr", name="thr")
    nc.vector.scalar_tensor_tensor(thr[:], acc2[:], -0.5, uhadd[:],
                                   op0=ALU.mult, op1=ALU.add)

    lrelu_op = dict(op0=ALU.mult, op1=ALU.max)

    def lrelu_into(dst, src, accum=None):
        nc.vector.scalar_tensor_tensor(dst, src, 0.01, src, accum_out=accum,
                                       **lrelu_op)

    def ec_gather(Qt, payload_sb, tagn):
        """top-3 idx from Q tiles -> wrapped idx -> gathered [128, 3*512].

        Rank-major: gathered col 512*l + i holds payload[:, nbr_l(i)] for node
        i = 128t+16c+p (the replication matmul reorders idx cols to l-major).
        """
        ts3 = seq_ps(3, N)
        for t in range(NT):
            m8 = pwork.tile([P, 8], F32, tag="m8g", name="m8g")
            nc.vector.max(m8[:], Qt[t])
            i8 = pwork.tile([P, 8], U32, tag="i8g", name="i8g")
            nc.vector.max_index(i8[:], m8[:], Qt[t])
            i8f = pwork.tile([P, 8], F32, tag="i8f", name="i8f")
            nc.vector.tensor_copy(i8f[:], i8[:])
            nc.tensor.transpose(ts3[0:3, 128 * t:128 * (t + 1)], i8f[:, 1:4], eye)
        ts3s = pwork.tile([3, N], F32, tag="ts3s", name="ts3s", padded_shape=[128, N])
        copy_ps(ts3s[:], ts3[0:3, :])
        wrap_ps = seq_ps(16, 96)
        for t in range(NT):
            for c in range(8):
                nc.tensor.transpose(
                    wrap_ps[0:16, 24 * t + 3 * c:24 * t + 3 * c + 3],
                    ts3s[0:3, 128 * t + 16 * c:128 * t + 16 * (c + 1)],
                    eye[0:3, 0:3])
        wrap16 = pwork.tile([16, 96], F32, tag="w16", name="w16",
                            padded_shape=[128, 96])
        copy_ps(wrap16[:], wrap_ps[0:16, :])
        # replicate to all 8 partition groups AND permute cols to l-major
        rep_ps = seq_ps(P, 96)
        nc.tensor.matmul(
            rep_ps[:], rep16,
            wrap16[:].rearrange("p (t c l) -> p l t c", t=NT, c=8, l=3),
            start=True, stop=True)
        wrap128 = pwork.tile([P, 96], U16, tag="w128", name="w128")
        nc.vector.tensor_copy(wrap128[:], rep_ps[:])
        gath = pbig.tile([P, 1536], F32, tag="gath", name="gath")
        for l in range(3):
            nc.gpsimd.indirect_copy(gath[:, 512 * l:512 * (l + 1)],
                                    payload_sb[:], wrap128[:, 32 * l:32 * (l + 1)],
                                    i_know_ap_gather_is_preferred=True)
        return gath

    zpack = [pst.tile([P, 12], F32, tag=f"zpack{g}", name=f"zpack{g}")
             for g in range(GPC)]

    # ---- EC1/EC2/EC3 chain (independent of TAG masks) ----
    yTs = [None] * GPC
    for g in range(GPC if not SKIP_EC else 0):
        a1_ps = seq_ps(P, N)
        nc.tensor.matmul(a1_ps[:], ec1_a, xgT[g][0:F, 0:N], start=True, stop=True)
        a1 = pwork.tile([P, N], F32, tag="a1", name="a1")
        nc.scalar.activation(a1[:], a1_ps[:], ACTF.Identity, bias=ec1_b1)
        g1_ps = seq_ps(P, N)
        nc.tensor.matmul(g1_ps[:], ec1_g, xgT[g][0:F, 0:N], start=True, stop=True)
        g1 = pwork.tile([P, N], F32, tag="g1", name="g1")
        copy_ps(g1[:], g1_ps[:])

        gath = ec_gather([q[:] for q in Q[g]], g1, f"e1{g}")
        m_ps = [quad_ps() for _ in range(3)]
        for l in range(3):
            hid = pbig.tile([P, N], F32, tag="hid", name="hid", bufs=2)
            nc.vector.tensor_tensor(hid[:], gath[:, 512 * l:512 * (l + 1)],
                                    a1[:], op=ALU.add)
            lrelu_into(hid[:], hid[:])
            nc.tensor.matmul(m_ps[l][:], ec1_w2, hid[:], start=True, stop=True)
        # max over ranks straight out of PSUM, bias after (max commutes w/ +b)
        mx = pwork.tile([P, N], F32, tag="mx", name="mx")
        nc.scalar.copy(mx[:], m_ps[0][:])
        nc.vector.tensor_tensor(mx[:], mx[:], m_ps[1][:], op=ALU.max)
        nc.vector.tensor_tensor(mx[:], mx[:], m_ps[2][:], op=ALU.max)
        mxb = pwork.tile([P, N], F32, tag="mxb", name="mxb")
        nc.scalar.activation(mxb[:], mx[:], ACTF.Identity, bias=ec1_b2)
        yT = pyT.tile([P, N], F32, tag="yT", name="yT")
        lrelu_into(yT[:], mxb[:], accum=zpack[g][:, 6:7])
        nc.vector.tensor_reduce(zpack[g][:, 9:10], yT[:], axis=AXX, op=ALU.max)
        yTs[g] = yT

    for f in range(2 if not (SKIP_EC or SKIP_EC23) else 0):
        for g in range(GPC):
            yT = yTs[g]
            y2 = pwork.tile([P, N], F32, tag="y2", name="y2")
            nc.vector.tensor_scalar(y2[:], yT[:], 2.0, 0.0, op0=ALU.mult)
            nysq = pwork.tile([P, N], F32, tag="nysq", name="nysq")
            nc.vector.scalar_tensor_tensor(nysq[:], yT[:], -2.0, yT[:],
                                           op0=ALU.mult, op1=ALU.mult)
            gy_ps = [quad_ps() for _ in range(NT)]
            for t in range(NT):
                nc.tensor.matmul(gy_ps[t][:], y2[:, 128 * t:128 * (t + 1)],
                                 y2[:], start=True, stop=False)
                nc.tensor.matmul(gy_ps[t][:], onesf[:], nysq[:],
                                 start=False, stop=True)
            gf_ps = seq_ps(P, N)
            nc.tensor.matmul(gf_ps[:], ecf_g[f], yT[:], start=True, stop=True)
            gf = pwork.tile([P, N], F32, tag="gf", name="gf")
            copy_ps(gf[:], gf_ps[:])
            af_ps = seq_ps(P, N)
            nc.tensor.matmul(af_ps[:], ecf_a[f], yT[:], start=True, stop=True)
            af = pwork.tile([P, N], F32, tag="af", name="af")
            nc.scalar.activation(af[:], af_ps[:], ACTF.Identity, bias=ecf_b[f])

            Q2 = []
            for t in range(NT):
                q2 = pq2.tile([P, N], F32, tag="Q2", name="Q2")
                copy_ps(q2[:], gy_ps[t][:])
                Q2.append(q2)
            gath2 = ec_gather([q[:] for q in Q2], gf, f"e{f+2}{g}")
            mx2 = pwork.tile([P, N], F32, tag="mx2", name="mx2")
            nc.vector.tensor_tensor(mx2[:], gath2[:, 0:512], gath2[:, 512:1024],
                                    op=ALU.max)
            nc.vector.tensor_tensor(mx2[:], mx2[:], gath2[:, 1024:1536],
                                    op=ALU.max)
            nc.vector.tensor_tensor(mx2[:], mx2[:], af[:], op=ALU.add)
            yT_new = pyT.tile([P, N], F32, tag="yT", name="yT")
            lrelu_into(yT_new[:], mx2[:], accum=zpack[g][:, 7 + f:8 + f])
            nc.vector.tensor_reduce(zpack[g][:, 10 + f:11 + f], yT_new[:],
                                    axis=AXX, op=ALU.max)
            yTs[g] = yT_new

    # ---- maskT (fused K=13) + TAG ----
    _cp[1] = True  # DVE has headroom from here on; alternate copies
    maskTs = [None] * GPC
    hTs = [None] * GPC
    hnms = [None] * GPC
    for g in range(GPC if not SKIP_TAG else 0):
        # negthr row -> xgT row 12 (cols 0:N), then T = 2G - n_j - thr_i >= 0
        un2 = pwork.tile([P, NT], F32, tag="un2", name="un2")
        nc.vector.tensor_scalar(un2[:], thr[:, 4 * g:4 * g + NT], -1.0, 0.0,
                                op0=ALU.mult)
        unps = seq_ps(1, N)
        for t in range(NT):
            nc.tensor.transpose(unps[0:1, 128 * t:128 * (t + 1)], un2[:, t:t + 1],
                                eye)
        copy_ps(junk_d[0:1, 0:N], unps[0:1, :])
        # partition 0 -> partition 12: SBUF->SBUF DMA (engines can't cross
        # partitions)
        dma(xps[g][12:13, 0:N], junk_d[0:1, 0:N])
        maskT = []
        for t in range(NT):
            tps = quad_ps()
            nc.tensor.matmul(tps[:], xgR[g][0:13, 128 * t:128 * (t + 1)],
                             xgT[g][0:13, 0:N], start=True, stop=True)
            mt = pmask.tile([P, N], F32R, tag="maskT", name="maskT")
            nc.vector.tensor_scalar(mt[:], tps[:], 0.0, 0.0, op0=ALU.is_ge)
            maskT.append(mt)

        maskTs[g] = maskT
        hTs[g] = xgT6r[g][:]
        hnms[g] = xnmr[g]

    for li in range(3):
        for g in range(GPC if not SKIP_TAG else 0):
            fin = F if li == 0 else W
            maskT = maskTs[g]
            hT = hTs[g]
            hnm = hnms[g]

            def hnm_sl(t, fin):
                return hnm[:, fin * t:fin * (t + 1)]
            def tag_ps(pp, nn, dt=F32):
                return quad_ps(pp, nn, dt)
            # u1T[f,i] = sum_j h[j,f] M[i,j]  (fp32r)
            u1T_ps = tag_ps(fin, N)
            for jc in range(NT):
                nc.tensor.matmul(u1T_ps[0:fin, :], hnm_sl(jc, fin), maskT[jc][:],
                                 start=(jc == 0), stop=(jc == NT - 1))
            u1T = pwork.tile([fin, N], F32R, tag="u1T", name="u1T",
                             padded_shape=[128, N])
            copy_ps(u1T[:], u1T_ps[0:fin, :])
            # u1 node-major via PE transpose of u1T
            u1n_ps = tag_ps(P, 4 * fin, F32R)
            for t in range(NT):
                nc.tensor.transpose(u1n_ps[0:P, fin * t:fin * (t + 1)],
                                    u1T[0:fin, 128 * t:128 * (t + 1)],
                                    eyer[0:fin, 0:fin])
            u1n = phn.tile([P, 4 * fin], F32R, tag="u1n", name="u1n")
            copy_ps(u1n[:], u1n_ps[0:P, 0:4 * fin])
            u2T_ps = tag_ps(fin, N)
            for jc in range(NT):
                nc.tensor.matmul(u2T_ps[0:fin, :],
                                 u1n[:, fin * jc:fin * (jc + 1)], maskT[jc][:],
                                 start=(jc == 0), stop=(jc == NT - 1))
            u2T = pwork.tile([fin, N], F32R, tag="u2T", name="u2T",
                             padded_shape=[128, N])
            copy_ps(u2T[:], u2T_ps[0:fin, :])
            oT_ps = tag_ps(P, N)
            nc.tensor.matmul(oT_ps[:], tagw[li][0], hT, start=True, stop=False)
            nc.tensor.matmul(oT_ps[:], tagw[li][1], u1T[:], start=False, stop=False)
            nc.tensor.matmul(oT_ps[:], tagw[li][2], u2T[:], start=False, stop=True)
            sT = pwork.tile([P, N], F32, tag="sT", name="sT")
            nc.scalar.activation(sT[:], oT_ps[:], ACTF.Identity,
                                 bias=tagb[li])
            hT_new = phT.tile([P, N], F32R, tag="hT", name="hT")
            lrelu_into(hT_new[:], sT[:], accum=zpack[g][:, 2 * li:2 * li + 1])
            nc.vector.tensor_reduce(zpack[g][:, 2 * li + 1:2 * li + 2], hT_new[:],
                                    axis=AXX, op=ALU.max)
            hTs[g] = hT_new[:]
            if li < 2:
                hps = tag_ps(P, N, F32R)
                for t in range(NT):
                    nc.tensor.transpose(hps[0:P, 128 * t:128 * (t + 1)],
                                        hT_new[:, 128 * t:128 * (t + 1)], eyer)
                hh = phm.tile([P, N], F32R, tag="hnm", name="hnm")
                copy_ps(hh[:], hps[:])
                hnms[g] = hh

    # ---- pools -> z0 row ----
    for g in range(GPC if not SKIP_TAG else 0):
        means_a = zpack[g][:].rearrange("p (a b) -> p a b", a=6, b=2)[:, 0:3, 0:1]
        nc.vector.tensor_scalar(means_a, means_a, 1.0 / N, 0.0, op0=ALU.mult)
        nc.vector.tensor_scalar(zpack[g][:, 6:9], zpack[g][:, 6:9], 1.0 / N, 0.0,
                                op0=ALU.mult)
        zr_ps = seq_ps(12, P)
        nc.tensor.transpose(zr_ps[0:12, 0:P], zpack[g][:], eye)
        zrow = pwork.tile([12, P], F32, tag="zrow", name="zrow",
                          padded_shape=[128, P])
        copy_ps(zrow[:], zr_ps[0:12, 0:P])
        dma(io['z0loc'][g].rearrange("(a b) -> a b", a=12), zrow[:])

    if skip_head:
        return

    # ---- head: one AllGather, then replicated fp16 lin stack ----
    # prefetch the first plinw.bufs W chunks during the graph stack; later
    # chunks are issued after z0s so their buffer-waits can't head-of-line
    # block the collective path on the DMA queue.
    NCHUNK = LIN_D * (12 // LIN_CHUNK)

    def lin_dma(idx):
        li, half = divmod(idx, 12 // LIN_CHUNK)
        wt = plinw.tile([P, LIN_CHUNK * DIM2], F16, tag="linw", name="linw")
        dma(wt[:].rearrange("p (k m) -> p k m", k=LIN_CHUNK),
            io['linW'][li, LIN_CHUNK * half:LIN_CHUNK * (half + 1)]
            .rearrange("k p m -> p k m"))
        return wt

    lin_chunks = [lin_dma(i) for i in range(2)]
    cores = list(range(NCORES))
    nc.gpsimd.collective_compute(
        "AllGather", ALU.bypass, replica_groups=[cores],
        ins=[io['z0loc'][:]], outs=[io['z0all'][:]])
    z0s = pst.tile([32, DIM2], F32, tag="z0s", name="z0s", padded_shape=[128, DIM2])
    dma(z0s[:], io['z0all'][:])
    for i in range(2, NCHUNK):
        li_, half_ = divmod(i, 12 // LIN_CHUNK)
        wt_ = plinw.tile([P, LIN_CHUNK * DIM2], F16, tag="linw", name="linw")
        nc.gpsimd.dma_start(
            wt_[:].rearrange("p (k m) -> p k m", k=LIN_CHUNK),
            io['linW'][li_, LIN_CHUNK * half_:LIN_CHUNK * (half_ + 1)]
            .rearrange("k p m -> p k m"))
        lin_chunks.append(wt_)
    if 'z0dump' in io:
        dma(io['z0dump'][:], z0s[:])

    zT = []
    spack = pst.tile([P, 12], F32, tag="spack", name="spack")
    qpack = pst.tile([P, 12], F32, tag="qpack", name="qpack")
    for t in range(12):
        zt_ps = seq_ps(P, 32)
        nc.tensor.transpose(zt_ps[0:P, 0:32], z0s[:, 128 * t:128 * (t + 1)],
                            eye[0:32, 0:32])
        zt = pst.tile([P, 32], F32, tag=f"zT{t}", name=f"zT{t}")
        copy_ps(zt[:], zt_ps[0:P, 0:32])
        zT.append(zt)
        nc.vector.tensor_reduce(spack[:, t:t + 1], zt[:], axis=AXX, op=ALU.add)
        nc.scalar.activation(junk_a[:, 0:32], zt[:], ACTF.Square,
                             accum_out=qpack[:, t:t + 1])
    mu = pst.tile([P, 12], F32, tag="mu", name="mu")
    nc.vector.tensor_scalar(mu[:], spack[:], 1.0 / 32, 0.0, op0=ALU.mult)
    m2 = pst.tile([P, 12], F32, tag="m2", name="m2")
    nc.vector.tensor_tensor(m2[:], mu[:], mu[:], op=ALU.mult)
    var = pst.tile([P, 12], F32, tag="var", name="var")
    nc.vector.scalar_tensor_tensor(var[:], qpack[:], 1.0 / 32, m2[:],
                                   op0=ALU.mult, op1=ALU.subtract)
    eps_col = pst.tile([P, 1], F32, tag="eps_col", name="eps_col")
    nc.any.memset(eps_col[:], 1e-5)
    sd = pst.tile([P, 12], F32, tag="sd", name="sd")
    nc.scalar.activation(sd[:], var[:], ACTF.Sqrt, bias=eps_col[:])
    inv = pst.tile([P, 12], F32, tag="inv", name="inv")
    nc.vector.reciprocal(inv[:], sd[:])
    gam = wslice('bn_scale')
    bet = wslice('bn_shift')
    sc = pst.tile([P, 12], F32, tag="sc", name="sc")
    nc.vector.tensor_tensor(sc[:], inv[:], gam, op=ALU.mult)
    bi = pst.tile([P, 12], F32, tag="bi", name="bi")
    nc.vector.tensor_tensor(bi[:], mu[:], sc[:], op=ALU.mult)
    nc.vector.tensor_tensor(bi[:], bet, bi[:], op=ALU.subtract)
    zcur = []
    for t in range(12):
        zc = pst.tile([P, 32], F16, tag=f"zc{t}", name=f"zc{t}")
        nc.scalar.activation(zc[:], zT[t][:], ACTF.Identity,
                             bias=bi[:, t:t + 1], scale=sc[:, t:t + 1])
        zcur.append(zc)
    if 'hdump' in io:
        for t in range(12):
            dma(io['hdump'][0, t], zcur[t][:])

    linB = wslice('linB')
    for li in range(LIN_D):
        # z-stationary: out[b, m] in three [32, 512] psum banks (one group
        # per bank), weights are the moving operand streamed in k-chunks.
        pz = [psh.tile([32, N], F32, tag=f"pz{s}", name=f"pz{s}")
              for s in range(3)]
        for half in range(12 // LIN_CHUNK):
            wt = lin_chunks[li * (12 // LIN_CHUNK) + half]
            for kk in range(LIN_CHUNK):
                kt = LIN_CHUNK * half + kk
                wsl = wt[:, DIM2 * kk:DIM2 * (kk + 1)]
                for s in range(3):
                    nc.tensor.matmul(
                        pz[s][:], zcur[kt][:], wsl[:, 512 * s:512 * (s + 1)],
                        start=(kt == 0), stop=(kt == 11))
        zrow = pst.tile([32, DIM2], F32, tag="zrowh",
                        name="zrowh", padded_shape=[128, DIM2])
        for s in range(3):
            copy_ps(zrow[:, 512 * s:512 * (s + 1)], pz[s][:])
        znew = []
        for mt in range(12):
            ztp = seq_ps(P, 32)
            nc.tensor.transpose(ztp[0:P, 0:32],
                                zrow[0:32, 128 * mt:128 * (mt + 1)],
                                eye[0:32, 0:32])
            bcol = linB[:, 12 * li + mt:12 * li + mt + 1]
            zb = pst.tile([P, 32], F32, tag=f"zb{li % 2}_{mt}",
                          name=f"zb{li % 2}_{mt}")
            nc.scalar.activation(zb[:], ztp[0:P, 0:32], ACTF.Identity, bias=bcol)
            zc = pst.tile([P, 32], F16, tag=f"zn{li % 2}_{mt}",
                          name=f"zn{li % 2}_{mt}")
            nc.vector.scalar_tensor_tensor(zc[:], zb[:], 0.01, zb[:], **lrelu_op)
            znew.append(zc)
        if 'hdump' in io and li == 0:
            for t in range(12):
                dma(io['hdump'][1, t], znew[t][:])
        zcur = znew

    outW16 = pst.tile([P, 12], F16, tag="outW16", name="outW16")
    dma(outW16[:], io['outW16'][:])
    outb_sb = wslice('outb', 1)
    ps_out = seq_ps(1, 32)
    for k in range(12):
        nc.tensor.matmul(ps_out[:], outW16[:, k:k + 1], zcur[k][:],
                         start=(k == 0), stop=(k == 11))
    o32 = pwork.tile([1, 32], F32, tag="o32", name="o32", padded_shape=[128, 32])
    nc.scalar.activation(o32[:], ps_out[:], ACTF.Identity, bias=outb_sb)
    dma(io['out32'][:], o32[:])


def build_nc(skip_head=False):
    nc = bacc.Bacc("TRN2", target_bir_lowering=False, debug=False,
                   num_devices=NCORES)
    io = {}

    io['xpack'] = nc.dram_tensor('xpack', [GPC, 128, 2 * N + 24], F32,
                                 kind="ExternalInput").ap()
    io['wpack'] = nc.dram_tensor('wpack', [128, WPACK_COLS], F32,
                                 kind="ExternalInput").ap()
    io['linW'] = nc.dram_tensor('linW', [LIN_D, 12, 128, DIM2], F16,
                                kind="ExternalInput").ap()
    io['outW16'] = nc.dram_tensor('outW16', [128, 12], F16,
                                  kind="ExternalInput").ap()

    io['z0loc'] = nc.dram_tensor(
        "z0loc", [GPC, DIM2], F32,
        kind="ExternalOutput" if skip_head else "Internal").ap()
    io['out32'] = nc.dram_tensor("out32", [1, 32], F32,
                                 kind="ExternalOutput").ap()
    if not skip_head:
        io['z0all'] = nc.dram_tensor("z0all", [B, DIM2], F32,
                                     addr_space="Shared").ap()

    with tile.TileContext(nc) as tc:
        core_program(tc, io, skip_head=skip_head)
    nc.compile()
    return nc


_CACHED = {}
_DEBUG_IO = None


def kernel(**inputs) -> np.ndarray:
    from concourse.bass_utils import run_bass_kernel_spmd
    if 'nc' not in _CACHED:
        _CACHED['nc'] = build_nc()
    nc = _CACHED['nc']
    in_maps = []
    for c in range(NCORES):
        d = prep_host(inputs, c)
        in_maps.append({k: np.ascontiguousarray(v) for k, v in d.items()})
    res = run_bass_kernel_spmd(nc, in_maps, core_ids=list(range(NCORES)),
                               trace=bool(os.environ.get("KBENCH_TRACE")))
    _CACHED['last'] = res
    return res.results[0]['out32'].reshape(-1).astype(np.float32)


if __name__ == "__main__":
    data = dict(np.load('/root/problem/inputs.npz'))
    out = kernel(**data)
    print("kernel out:", out[:5])


# revision 49
# speedup vs baseline: 1.0457x; 1.0457x over previous
"""Trainium2 Bass kernel for nn_EnsembleNet3 (gnn_message_passing).

Self-contained: takes full inputs (as produced by setup_inputs()), shards the
B=32 graph dim over 8 NeuronCores (4 graphs/core), runs the graph stack fully
on-device. The [B,1536] head is replicated on every core after a single
AllGather of the pooled features (BatchNorm couples graphs); lin weights
stream from HBM in fp16, W-stationary.

Per graph (N=512 nodes):
- kNN-100 for TAGConv: per-row threshold via count-secant iteration on
  Q[i,j] = 2*G[i,j] - n_j (same per-row order as -dist; Q row max is self),
  finished by an exact masked max-8 endgame extracting the 101st and 102nd
  largest; the mask threshold sits mid-gap between them so the fp32 maskT
  recompute (fused K=13 matmul with the threshold row folded into xgT row 12)
  is robust to rounding. Adjacency applied as a dense 0/1 mask matmul on PE
  (float32r) with host-folded hop weights:
  out = h@W~0 + (Mh)@W~1 + (M^2 h)@W~2,  M = mask incl self.
  Node-major M@h obtained by PE-transposing the feature-major aggregate.
- EdgeConv k=3: top-3 indices via max8+max_index on Q; indices marshalled into
  the GPSIMD wrapped-16 layout with PE piece-transposes + a replication
  matmul, gather via indirect_copy. MLPs decomposed as a_i + g_j so only g is
  gathered; max-aggregation commutes with leaky_relu. leaky_relu computed as
  max(x, 0.01x) in one fused DVE op (with free mean-pool accumulation).
"""
import os
from contextlib import ExitStack

import numpy as np

import concourse.bass as bass
import concourse.bacc as bacc
import concourse.tile as tile
from concourse import mybir
from concourse._compat import with_exitstack

F32 = mybir.dt.float32
F32R = mybir.dt.float32r
F16 = mybir.dt.float16
U16 = mybir.dt.uint16
U32 = mybir.dt.uint32
U8 = mybir.dt.uint8
ALU = mybir.AluOpType
ACTF = mybir.ActivationFunctionType
AXX = mybir.AxisListType.X

B, N, F, W = 32, 512, 6, 128
NT = N // 128
GPC = 4
NCORES = 8
K101 = 101
SEL_ITERS = 11
SEL_TARGET = float(K101 + 4)
U_LO, U_HI = -64.0, 64.0
DIM2 = 1536
LIN_D = 5
DVE_COLS = 8     # selection count passes: cols < DVE_COLS on DVE, rest on ACT
LIN_CHUNK = 2    # k-tiles per streamed lin_W chunk (6 chunks per layer)


def _fold_tag(Wk, b):
    W0, W1, W2 = Wk[0], Wk[1], Wk[2]
    c1, c2 = 1.0 / 100.0, 1.0 / 10000.0
    return (
        (W0 - W1 * c1 + W2 * c2).astype(np.float32),
        (W1 * c1 - 2.0 * W2 * c2).astype(np.float32),
        (W2 * c2).astype(np.float32),
        b.astype(np.float32),
    )


def prep_host(inputs, core):
    inp = {k: np.asarray(v) for k, v in inputs.items()}
    x = inp['x'].astype(np.float32).reshape(B, N, F)
    xs = x[GPC * core:GPC * (core + 1)]
    f32 = np.float32

    # --- per-graph input pack [128, 1048]: xgT | xgR | xnm ---
    xt = xs.transpose(0, 2, 1)
    xpack = np.zeros((GPC, 128, 2 * N + 24), f32)
    xpack[:, 0:F, 0:N] = xt
    xpack[:, F:2 * F, 0:N] = 1.0
    xpack[:, 0:F, N:2 * N] = 2.0 * xt
    xpack[:, F:2 * F, N:2 * N] = -(xt * xt)
    xpack[:, 12, N:2 * N] = 1.0
    for t in range(NT):
        xpack[:, :, 2 * N + F * t:2 * N + F * (t + 1)] = xs[:, 128 * t:128 * (t + 1), :]

    # --- const pack [128, cols] ---
    cols = {}
    blobs = []
    off = 0

    def put(name, arr2d):
        nonlocal off
        a = np.asarray(arr2d, f32)
        pad = np.zeros((128, a.shape[1]), f32)
        pad[:a.shape[0]] = a
        cols[name] = (off, a.shape[1])
        blobs.append(pad)
        off += a.shape[1]

    put('eye', np.eye(128, dtype=f32))
    put('iota8', np.broadcast_to(np.arange(8, dtype=f32), (128, 8)))
    rep16 = np.zeros((16, 128), f32)
    for q in range(128):
        rep16[q % 16, q] = 1.0
    put('rep16', rep16)
    # fp32r-rounded weight block: [tagw0|tagw1|tagw2|ec1_a|ec1_g|ec1_w2|
    #                              ec2_a|ec2_g|ec3_a|ec3_g]
    for li, (Wk, bk) in enumerate(
            [(inp['tag1_W'], inp['tag1_b']),
             (inp['tag_W'][0], inp['tag_b'][0]),
             (inp['tag_W'][1], inp['tag_b'][1])]):
        w0, w1, w2, bb = _fold_tag(Wk, bk)
        put(f'tagw{li}', np.concatenate([w0, w1, w2], axis=1))
        put(f'tagb{li}', bb.reshape(128, 1))
    W1 = inp['p1_W1'].astype(f32)
    put('ec1_a', W1[:F] - W1[F:])
    put('ec1_g', W1[F:])
    put('ec1_w2', inp['p1_W2'].astype(f32))
    for f in range(2):
        Wf = inp['pf_W'][f].astype(f32)
        put(f'ec{f+2}_a', Wf[:W] - Wf[W:])
        put(f'ec{f+2}_g', Wf[W:])
    put('ec1_b1', inp['p1_b1'].astype(f32).reshape(128, 1))
    put('ec1_b2', inp['p1_b2'].astype(f32).reshape(128, 1))
    for f in range(2):
        put(f'ec{f+2}_b', inp['pf_b'][f].astype(f32).reshape(128, 1))
    put('bn_scale', inp['bn_gamma'].astype(f32).reshape(12, 128).T)
    put('bn_shift', inp['bn_beta'].astype(f32).reshape(12, 128).T)
    put('outb', np.full((1, 1), float(inp['out_b'][0]), f32))
    put('linB', inp['lin_b'].astype(f32).reshape(LIN_D * 12, 128).T)
    wpack = np.concatenate(blobs, axis=1)
    assert wpack.shape[1] == WPACK_COLS, (wpack.shape, off)
    assert all(cols[k] == WOFF[k] for k in cols), "WOFF mismatch"

    # --- lin weights fp16, W-stationary: [LIN_D, 12 ktile, 128, 1536] ---
    linW = inp['lin_W'].astype(np.float16).reshape(LIN_D, 12, 128, DIM2)
    d = {
        'xpack': np.ascontiguousarray(xpack),
        'wpack': np.ascontiguousarray(wpack),
        'linW': np.ascontiguousarray(linW),
        'outW16': np.ascontiguousarray(
            inp['out_W'].astype(np.float16).reshape(12, 128).T),
    }
    return d


def _woff_table():
    off = 0
    tab = {}

    def put(name, w):
        nonlocal off
        tab[name] = (off, w)
        off += w
    put('eye', 128); put('iota8', 8); put('rep16', 128)
    # fp32r block start
    tab['_r_begin'] = (off, 0)
    for li in range(3):
        put(f'tagw{li}', 384); put(f'tagb{li}', 1)
    tab['_r_end'] = (off, 0)
    put('ec1_a', 128); put('ec1_g', 128); put('ec1_w2', 128)
    for f in range(2):
        put(f'ec{f+2}_a', 128); put(f'ec{f+2}_g', 128)
    put('ec1_b1', 1); put('ec1_b2', 1)
    for f in range(2):
        put(f'ec{f+2}_b', 1)
    put('bn_scale', 12); put('bn_shift', 12)
    put('outb', 1)
    put('linB', LIN_D * 12)
    return tab, off


WOFF, WPACK_COLS = _woff_table()


@with_exitstack
def core_program(ctx: ExitStack, tc: tile.TileContext, io: dict, skip_head=False):
    nc = tc.nc
    P = 128
    SKIP_EC23 = bool(os.environ.get("K_SKIP_EC23"))
    SKIP_EC = bool(os.environ.get("K_SKIP_EC"))
    SKIP_TAG = bool(os.environ.get("K_SKIP_TAG"))

    const = ctx.enter_context(tc.tile_pool(name="const", bufs=1))
    pq = ctx.enter_context(tc.tile_pool(name="pq", bufs=16))
    pmask = ctx.enter_context(tc.tile_pool(name="pmask", bufs=16))
    pwork = ctx.enter_context(tc.tile_pool(name="pwork", bufs=1))
    pbig = ctx.enter_context(tc.tile_pool(name="pbig", bufs=1))
    phT = ctx.enter_context(tc.tile_pool(name="phT", bufs=4))
    pyT = ctx.enter_context(tc.tile_pool(name="pyT", bufs=2))
    phn = ctx.enter_context(tc.tile_pool(name="phn", bufs=2))
    phm = ctx.enter_context(tc.tile_pool(name="phm", bufs=4))
    pq2 = ctx.enter_context(tc.tile_pool(name="pq2", bufs=2))
    pst = ctx.enter_context(tc.tile_pool(name="pst", bufs=1))
    plinw = ctx.enter_context(tc.tile_pool(name="plinw", bufs=2))
    psq = ctx.enter_context(tc.tile_pool(name="psq", bufs=3, space="PSUM"))
    pss = ctx.enter_context(tc.tile_pool(name="pss", bufs=2, space="PSUM"))
    psh = ctx.enter_context(tc.tile_pool(name="psh", bufs=1, space="PSUM"))

    def quad_ps(pp=P, nn=N, dt=F32):
        return psq.tile([pp, nn], dt, tag="quad", name="quad")

    def seq_ps(pp, nn, dt=F32):
        return pss.tile([pp, nn], dt, tag="seq", name="seq")

    def dma(dst, src):
        nc.sync.dma_start(dst, src)

    _cp = [0, False]

    def copy_ps(dst, src):
        if _cp[1] and _cp[0] % 2 == 1:
            nc.vector.tensor_copy(dst, src)
        else:
            nc.scalar.copy(dst, src)
        _cp[0] += 1

    # ---- constants: one packed DMA ----
    wp = const.tile([P, WPACK_COLS], F32, tag="wpack", name="wpack")
    dma(wp[:], io['wpack'][:])

    def wslice(name, rows=128):
        o, w = WOFF[name]
        return wp[0:rows, o:o + w]

    eye = wslice('eye')
    iota8 = wslice('iota8')
    rep16 = wslice('rep16', 16)
    eyer = const.tile([P, P], F32R)
    nc.vector.tensor_copy(eyer[:], eye)
    onesf = const.tile([P, P], F32)
    nc.any.memset(onesf[:], 1.0)

    # fp32r copy of the weight block
    r0 = WOFF['_r_begin'][0]
    r1 = WOFF['_r_end'][0]
    wpr = const.tile([P, r1 - r0], F32R, tag="wpr", name="wpr")
    nc.vector.tensor_copy(wpr[:], wp[:, r0:r1])

    def wslice_r(name, rows=128):
        o, w = WOFF[name]
        return wpr[0:rows, o - r0:o - r0 + w]

    tagw, tagb = [], []
    for li in range(3):
        fin = F if li == 0 else W
        wt = wslice_r(f'tagw{li}', fin)
        tagw.append([wt[:, 128 * k:128 * (k + 1)] for k in range(3)])
        tagb.append(wslice(f'tagb{li}'))

    ec1_a = wslice('ec1_a', F)
    ec1_g = wslice('ec1_g', F)
    ec1_w2 = wslice('ec1_w2')
    ec1_b1 = wslice('ec1_b1')
    ec1_b2 = wslice('ec1_b2')
    ecf_a = [wslice('ec2_a'), wslice('ec3_a')]
    ecf_g = [wslice('ec2_g'), wslice('ec3_g')]
    ecf_b = [wslice('ec2_b'), wslice('ec3_b')]

    # ---- inputs per graph: one packed DMA each ----
    xgT, xgR, xnm = [], [], []
    xps = []
    for g in range(GPC):
        xp = pst.tile([P, 2 * N + 24], F32, tag=f"xpack{g}", name=f"xpack{g}")
        dma(xp[:], io['xpack'][g])
        xps.append(xp)
        xgT.append(xp[:, 0:N])
        xgR.append(xp[:, N:2 * N])
        xnm.append(xp[:, 2 * N:2 * N + 24])

    # fp32r copies of x inputs used in f32r matmuls
    xgT6r, xnmr = [], []
    for g in range(GPC):
        xr = pst.tile([F, N], F32R, tag=f"xgT6r{g}", name=f"xgT6r{g}",
                      padded_shape=[128, N])
        nc.vector.tensor_copy(xr[:], xgT[g][0:F, 0:N])
        xgT6r.append(xr)
        xnr = pst.tile([P, 24], F32R, tag=f"xnmr{g}", name=f"xnmr{g}")
        nc.vector.tensor_copy(xnr[:], xnm[g])
        xnmr.append(xnr)

    # ---- Q = 2G - n_row via augmented matmul (K=12), fp32 exact ----
    Q = [[None] * NT for _ in range(GPC)]
    for g in range(GPC):
        gps = [quad_ps() for _ in range(NT)]
        for t in range(NT):
            nc.tensor.matmul(gps[t][:], xgT[g][0:12, 128 * t:128 * (t + 1)],
                             xgR[g][0:12, 0:N], start=True, stop=True)
        for t in range(NT):
            qt = pq.tile([P, N], F32, tag="Q", name="Q")
            copy_ps(qt[:], gps[t][:])
            Q[g][t] = qt

    # ---- lockstep count-secant selection ----
    NC16 = GPC * NT
    st_u = pst.tile([P, NC16], F32, tag="st_u", name="st_u")
    st_ul = pst.tile([P, NC16], F32, tag="st_ul", name="st_ul")
    st_uh = pst.tile([P, NC16], F32, tag="st_uh", name="st_uh")
    st_cl = pst.tile([P, NC16], F32, tag="st_cl", name="st_cl")
    st_ch = pst.tile([P, NC16], F32, tag="st_ch", name="st_ch")
    cnt = pst.tile([P, NC16], F32, tag="cnt", name="cnt")
    tmp_a = pst.tile([P, NC16], F32, tag="tmp_a", name="tmp_a")
    tmp_b = pst.tile([P, NC16], F32, tag="tmp_b", name="tmp_b")
    tmp_m = pst.tile([P, NC16], U8, tag="tmp_m", name="tmp_m")
    junk_d = pst.tile([P, N], F32, tag="junk_d", name="junk_d")
    junk_a = pst.tile([P, N], F32, tag="junk_a", name="junk_a")
    nc.any.memset(st_ul[:], U_HI)
    nc.any.memset(st_cl[:], 0.0)
    nc.any.memset(st_uh[:], U_LO)
    nc.any.memset(st_ch[:], float(N))
    nc.any.memset(st_u[:], U_HI + (U_LO - U_HI) * (SEL_TARGET / N))

    for it in range(SEL_ITERS):
        for g in range(GPC):
            for t in range(NT):
                col = 4 * g + t
                ucol = st_u[:, col:col + 1]
                ccol = cnt[:, col:col + 1]
                if col < DVE_COLS:
                    nc.vector.tensor_scalar(
                        junk_d[:], Q[g][t][:], ucol, 0.0,
                        op0=ALU.is_ge, op1=ALU.add, accum_out=ccol)
                else:
                    nc.scalar.activation(
                        junk_a[:], Q[g][t][:], ACTF.Sign,
                        bias=ucol, scale=-1.0, accum_out=ccol)
        # ACT cols: c = 256 - s/2
        nc.vector.tensor_scalar(
            cnt[:, DVE_COLS:NC16], cnt[:, DVE_COLS:NC16], -0.5, 256.0,
            op0=ALU.mult, op1=ALU.add)
        nc.vector.tensor_scalar(
            tmp_m[:], cnt[:], float(K101) - 0.5, 0.0, op0=ALU.is_ge)
        nc.vector.copy_predicated(st_uh[:], tmp_m[:], st_u[:])
        nc.vector.copy_predicated(st_ch[:], tmp_m[:], cnt[:])
        nc.vector.tensor_scalar(
            tmp_m[:], cnt[:], float(K101) - 0.5, 0.0, op0=ALU.is_lt)
        nc.vector.copy_predicated(st_ul[:], tmp_m[:], st_u[:])
        nc.vector.copy_predicated(st_cl[:], tmp_m[:], cnt[:])
        if it == SEL_ITERS - 1:
            break
        nc.vector.tensor_tensor(tmp_a[:], st_ch[:], st_cl[:], op=ALU.subtract)
        nc.vector.reciprocal(tmp_a[:], tmp_a[:])
        nc.vector.scalar_tensor_tensor(
            tmp_b[:], st_ch[:], -SEL_TARGET, tmp_a[:], op0=ALU.add, op1=ALU.mult)
        nc.vector.tensor_scalar(
            tmp_b[:], tmp_b[:], 0.05, 0.95, op0=ALU.max, op1=ALU.min)
        nc.vector.tensor_tensor(tmp_a[:], st_ul[:], st_uh[:], op=ALU.subtract)
        nc.vector.tensor_tensor(tmp_a[:], tmp_a[:], tmp_b[:], op=ALU.mult)
        nc.vector.tensor_tensor(st_u[:], st_uh[:], tmp_a[:], op=ALU.add)

    # ---- endgame: exact 101st + 102nd largest of each Q row ----
    # mask threshold sits mid-gap so the maskT fp32 recompute can't flip the
    # boundary neighbor. When pos==0 the 102nd value is below the uh bracket,
    # and uh itself is a valid lower mid-point.
    # acc[col] = -(u101 + u102) via a single two-rank mask (iota in
    # {pos-1, pos}); when pos==0 only u101 lands, patched with uh below.
    acc2 = pst.tile([P, NC16], F32, tag="acc2", name="acc2")
    posh = pst.tile([P, NC16], F32, tag="posh", name="posh")
    nc.vector.tensor_scalar(posh[:], st_ch[:], -float(K101) - 0.5, 0.0,
                            op0=ALU.add)
    for g in range(GPC):
        for t in range(NT):
            col = 4 * g + t
            zm = pwork.tile([P, N], F32, tag="zm", name="zm")
            nc.vector.tensor_scalar(
                zm[:], Q[g][t][:], st_uh[:, col:col + 1], -1e30,
                op0=ALU.is_lt, op1=ALU.mult)
            nc.vector.tensor_tensor(zm[:], zm[:], Q[g][t][:], op=ALU.subtract)
            m8 = pwork.tile([P, 8], F32, tag="m8e", name="m8e")
            nc.vector.max(m8[:], zm[:])
            d8 = pwork.tile([P, 8], F32, tag="d8", name="d8")
            nc.vector.tensor_tensor(
                d8[:], iota8,
                posh[:, col:col + 1].broadcast_to([P, 8]), op=ALU.subtract)
            msk8 = pwork.tile([P, 8], F32, tag="msk8", name="msk8")
            nc.vector.scalar_tensor_tensor(
                msk8[:], d8[:], 1.0, d8[:], op0=ALU.mult, op1=ALU.mult)
            nc.vector.tensor_scalar(msk8[:], msk8[:], 1.0, 0.0, op0=ALU.is_lt)
            j8 = pwork.tile([P, 8], F32, tag="j8", name="j8")
            nc.vector.scalar_tensor_tensor(
                j8[:], m8[:], 1.0, msk8[:], op0=ALU.mult, op1=ALU.mult,
                accum_out=acc2[:, col:col + 1])
    pos0 = pst.tile([P, NC16], U8, tag="pos0", name="pos0")
    nc.vector.tensor_scalar(pos0[:], posh[:], 0.0, 0.0, op0=ALU.is_lt)
    uhadd = pst.tile([P, NC16], F32, tag="uhadd", name="uhadd")
    nc.any.memset(uhadd[:], 0.0)
    nc.vector.copy_predicated(uhadd[:], pos0[:], st_uh[:])
    nc.vector.tensor_scalar(uhadd[:], uhadd[:], 0.5, 0.0, op0=ALU.mult)
    thr = pst.tile([P, NC16], F32, tag="thr", name="thr")
    nc.vector.scalar_tensor_tensor(thr[:], acc2[:], -0.5, uhadd[:],
                                   op0=ALU.mult, op1=ALU.add)

    lrelu_op = dict(op0=ALU.mult, op1=ALU.max)

    def lrelu_into(dst, src, accum=None):
        nc.vector.scalar_tensor_tensor(dst, src, 0.01, src, accum_out=accum,
                                       **lrelu_op)

    def ec_gather(Qt, payload_sb, tagn):
        """top-3 idx from Q tiles -> wrapped idx -> gathered [128, 3*512].

        Rank-major: gathered col 512*l + i holds payload[:, nbr_l(i)] for node
        i = 128t+16c+p (the replication matmul reorders idx cols to l-major).
        """
        ts3 = seq_ps(3, N)
        for t in range(NT):
            m8 = pwork.tile([P, 8], F32, tag="m8g", name="m8g")
            nc.vector.max(m8[:], Qt[t])
            i8 = pwork.tile([P, 8], U32, tag="i8g", name="i8g")
            nc.vector.max_index(i8[:], m8[:], Qt[t])
            i8f = pwork.tile([P, 8], F32, tag="i8f", name="i8f")
            nc.vector.tensor_copy(i8f[:], i8[:])
            nc.tensor.transpose(ts3[0:3, 128 * t:128 * (t + 1)], i8f[:, 1:4], eye)
        ts3s = pwork.tile([3, N], F32, tag="ts3s", name="ts3s", padded_shape=[128, N])
        copy_ps(ts3s[:], ts3[0:3, :])
        wrap_ps = seq_ps(16, 96)
        for t in range(NT):
            for c in range(8):
                nc.tensor.transpose(
                    wrap_ps[0:16, 24 * t + 3 * c:24 * t + 3 * c + 3],
                    ts3s[0:3, 128 * t + 16 * c:128 * t + 16 * (c + 1)],
                    eye[0:3, 0:3])
        wrap16 = pwork.tile([16, 96], F32, tag="w16", name="w16",
                            padded_shape=[128, 96])
        copy_ps(wrap16[:], wrap_ps[0:16, :])
        # replicate to all 8 partition groups AND permute cols to l-major
        rep_ps = seq_ps(P, 96)
        nc.tensor.matmul(
            rep_ps[:], rep16,
            wrap16[:].rearrange("p (t c l) -> p l t c", t=NT, c=8, l=3),
            start=True, stop=True)
        wrap128 = pwork.tile([P, 96], U16, tag="w128", name="w128")
        nc.vector.tensor_copy(wrap128[:], rep_ps[:])
        gath = pbig.tile([P, 1536], F32, tag="gath", name="gath")
        for l in range(3):
            nc.gpsimd.indirect_copy(gath[:, 512 * l:512 * (l + 1)],
                                    payload_sb[:], wrap128[:, 32 * l:32 * (l + 1)],
                                    i_know_ap_gather_is_preferred=True)
        return gath

    zpack = [pst.tile([P, 12], F32, tag=f"zpack{g}", name=f"zpack{g}")
             for g in range(GPC)]

    # ---- EC1/EC2/EC3 chain (independent of TAG masks) ----
    yTs = [None] * GPC
    for g in range(GPC if not SKIP_EC else 0):
        a1_ps = seq_ps(P, N)
        nc.tensor.matmul(a1_ps[:], ec1_a, xgT[g][0:F, 0:N], start=True, stop=True)
        a1 = pwork.tile([P, N], F32, tag="a1", name="a1")
        nc.scalar.activation(a1[:], a1_ps[:], ACTF.Identity, bias=ec1_b1)
        g1_ps = seq_ps(P, N)
        nc.tensor.matmul(g1_ps[:], ec1_g, xgT[g][0:F, 0:N], start=True, stop=True)
        g1 = pwork.tile([P, N], F32, tag="g1", name="g1")
        copy_ps(g1[:], g1_ps[:])

        gath = ec_gather([q[:] for q in Q[g]], g1, f"e1{g}")
        m_ps = [quad_ps() for _ in range(3)]
        for l in range(3):
            hid = pbig.tile([P, N], F32, tag="hid", name="hid", bufs=2)
            nc.vector.tensor_tensor(hid[:], gath[:, 512 * l:512 * (l + 1)],
                                    a1[:], op=ALU.add)
            lrelu_into(hid[:], hid[:])
            nc.tensor.matmul(m_ps[l][:], ec1_w2, hid[:], start=True, stop=True)
        # max over ranks straight out of PSUM, bias after (max commutes w/ +b)
        mx = pwork.tile([P, N], F32, tag="mx", name="mx")
        nc.scalar.copy(mx[:], m_ps[0][:])
        nc.vector.tensor_tensor(mx[:], mx[:], m_ps[1][:], op=ALU.max)
        nc.vector.tensor_tensor(mx[:], mx[:], m_ps[2][:], op=ALU.max)
        mxb = pwork.tile([P, N], F32, tag="mxb", name="mxb")
        nc.scalar.activation(mxb[:], mx[:], ACTF.Identity, bias=ec1_b2)
        yT = pyT.tile([P, N], F32, tag="yT", name="yT")
        lrelu_into(yT[:], mxb[:], accum=zpack[g][:, 6:7])
        nc.vector.tensor_reduce(zpack[g][:, 9:10], yT[:], axis=AXX, op=ALU.max)
        yTs[g] = yT

    for f in range(2 if not (SKIP_EC or SKIP_EC23) else 0):
        for g in range(GPC):
            yT = yTs[g]
            y2 = pwork.tile([P, N], F32, tag="y2", name="y2")
            nc.vector.tensor_scalar(y2[:], yT[:], 2.0, 0.0, op0=ALU.mult)
            nysq = pwork.tile([P, N], F32, tag="nysq", name="nysq")
            nc.vector.scalar_tensor_tensor(nysq[:], yT[:], -2.0, yT[:],
                                           op0=ALU.mult, op1=ALU.mult)
            gy_ps = [quad_ps() for _ in range(NT)]
            for t in range(NT):
                nc.tensor.matmul(gy_ps[t][:], y2[:, 128 * t:128 * (t + 1)],
                                 y2[:], start=True, stop=False)
                nc.tensor.matmul(gy_ps[t][:], onesf[:], nysq[:],
                                 start=False, stop=True)
            gf_ps = seq_ps(P, N)
            nc.tensor.matmul(gf_ps[:], ecf_g[f], yT[:], start=True, stop=True)
            gf = pwork.tile([P, N], F32, tag="gf", name="gf")
            copy_ps(gf[:], gf_ps[:])
            af_ps = seq_ps(P, N)
            nc.tensor.matmul(af_ps[:], ecf_a[f], yT[:], start=True, stop=True)
            af = pwork.tile([P, N], F32, tag="af", name="af")
            nc.scalar.activation(af[:], af_ps[:], ACTF.Identity, bias=ecf_b[f])

            Q2 = []
            for t in range(NT):
                q2 = pq2.tile([P, N], F32, tag="Q2", name="Q2")
                copy_ps(q2[:], gy_ps[t][:])
                Q2.append(q2)
            gath2 = ec_gather([q[:] for q in Q2], gf, f"e{f+2}{g}")
            mx2 = pwork.tile([P, N], F32, tag="mx2", name="mx2")
            nc.vector.tensor_tensor(mx2[:], gath2[:, 0:512], gath2[:, 512:1024],
                                    op=ALU.max)
            nc.vector.tensor_tensor(mx2[:], mx2[:], gath2[:, 1024:1536],
                                    op=ALU.max)
            nc.vector.tensor_tensor(mx2[:], mx2[:], af[:], op=ALU.add)
            yT_new = pyT.tile([P, N], F32, tag="yT", name="yT")
            lrelu_into(yT_new[:], mx2[:], accum=zpack[g][:, 7 + f:8 + f])
            nc.vector.tensor_reduce(zpack[g][:, 10 + f:11 + f], yT_new[:],
                                    axis=AXX, op=ALU.max)
            yTs[g] = yT_new

    # ---- maskT (fused K=13) + TAG ----
    _cp[1] = True  # DVE has headroom from here on; alternate copies
    maskTs = [None] * GPC
    hTs = [None] * GPC
    hnms = [None] * GPC
    for g in range(GPC if not SKIP_TAG else 0):
        # negthr row -> xgT row 12 (cols 0:N), then T = 2G - n_j - thr_i >= 0
        un2 = pwork.tile([P, NT], F32, tag="un2", name="un2")
        nc.vector.tensor_scalar(un2[:], thr[:, 4 * g:4 * g + NT], -1.0, 0.0,
                                op0=ALU.mult)
        unps = seq_ps(1, N)
        for t in range(NT):
            nc.tensor.transpose(unps[0:1, 128 * t:128 * (t + 1)], un2[:, t:t + 1],
                                eye)
        copy_ps(junk_d[0:1, 0:N], unps[0:1, :])
        # partition 0 -> partition 12: SBUF->SBUF DMA (engines can't cross
        # partitions)
        dma(xps[g][12:13, 0:N], junk_d[0:1, 0:N])
        maskT = []
        for t in range(NT):
            tps = quad_ps()
            nc.tensor.matmul(tps[:], xgR[g][0:13, 128 * t:128 * (t + 1)],
                             xgT[g][0:13, 0:N], start=True, stop=True)
            mt = pmask.tile([P, N], F32R, tag="maskT", name="maskT")
            nc.vector.tensor_scalar(mt[:], tps[:], 0.0, 0.0, op0=ALU.is_ge)
            maskT.append(mt)

        maskTs[g] = maskT
        hTs[g] = xgT6r[g][:]
        hnms[g] = xnmr[g]

    for li in range(3):
        for g in range(GPC if not SKIP_TAG else 0):
            fin = F if li == 0 else W
            maskT = maskTs[g]
            hT = hTs[g]
            hnm = hnms[g]

            def hnm_sl(t, fin):
                return hnm[:, fin * t:fin * (t + 1)]
            def tag_ps(pp, nn, dt=F32):
                return quad_ps(pp, nn, dt)
            # u1T[f,i] = sum_j h[j,f] M[i,j]  (fp32r)
            u1T_ps = tag_ps(fin, N)
            for jc in range(NT):
                nc.tensor.matmul(u1T_ps[0:fin, :], hnm_sl(jc, fin), maskT[jc][:],
                                 start=(jc == 0), stop=(jc == NT - 1))
            u1T = pwork.tile([fin, N], F32R, tag="u1T", name="u1T",
                             padded_shape=[128, N])
            copy_ps(u1T[:], u1T_ps[0:fin, :])
            # u1 node-major via PE transpose of u1T
            u1n_ps = tag_ps(P, 4 * fin, F32R)
            for t in range(NT):
                nc.tensor.transpose(u1n_ps[0:P, fin * t:fin * (t + 1)],
                                    u1T[0:fin, 128 * t:128 * (t + 1)],
                                    eyer[0:fin, 0:fin])
            u1n = phn.tile([P, 4 * fin], F32R, tag="u1n", name="u1n")
            copy_ps(u1n[:], u1n_ps[0:P, 0:4 * fin])
            u2T_ps = tag_ps(fin, N)
            for jc in range(NT):
                nc.tensor.matmul(u2T_ps[0:fin, :],
                                 u1n[:, fin * jc:fin * (jc + 1)], maskT[jc][:],
                                 start=(jc == 0), stop=(jc == NT - 1))
            u2T = pwork.tile([fin, N], F32R, tag="u2T", name="u2T",
                             padded_shape=[128, N])
            copy_ps(u2T[:], u2T_ps[0:fin, :])
            oT_ps = tag_ps(P, N)
            nc.tensor.matmul(oT_ps[:], tagw[li][0], hT, start=True, stop=False)
            nc.tensor.matmul(oT_ps[:], tagw[li][1], u1T[:], start=False, stop=False)
            nc.tensor.matmul(oT_ps[:], tagw[li][2], u2T[:], start=False, stop=True)
            sT = pwork.tile([P, N], F32, tag="sT", name="sT")
            nc.scalar.activation(sT[:], oT_ps[:], ACTF.Identity,
                                 bias=tagb[li])
            hT_new = phT.tile([P, N], F32R, tag="hT", name="hT")
            lrelu_into(hT_new[:], sT[:], accum=zpack[g][:, 2 * li:2 * li + 1])
            nc.vector.tensor_reduce(zpack[g][:, 2 * li + 1:2 * li + 2], hT_new[:],
                                    axis=AXX, op=ALU.max)
            hTs[g] = hT_new[:]
            if li < 2:
                hps = tag_ps(P, N, F32R)
                for t in range(NT):
                    nc.tensor.transpose(hps[0:P, 128 * t:128 * (t + 1)],
                                        hT_new[:, 128 * t:128 * (t + 1)], eyer)
                hh = phm.tile([P, N], F32R, tag="hnm", name="hnm")
                copy_ps(hh[:], hps[:])
                hnms[g] = hh

    # ---- pools -> z0 row ----
    for g in range(GPC if not SKIP_TAG else 0):
        means_a = zpack[g][:].rearrange("p (a b) -> p a b", a=6, b=2)[:, 0:3, 0:1]
        nc.vector.tensor_scalar(means_a, means_a, 1.0 / N, 0.0, op0=ALU.mult)
        nc.vector.tensor_scalar(zpack[g][:, 6:9], zpack[g][:, 6:9], 1.0 / N, 0.0,
                                op0=ALU.mult)
        zr_ps = seq_ps(12, P)
        nc.tensor.transpose(zr_ps[0:12, 0:P], zpack[g][:], eye)
        zrow = pwork.tile([12, P], F32, tag="zrow", name="zrow",
                          padded_shape=[128, P])
        copy_ps(zrow[:], zr_ps[0:12, 0:P])
        dma(io['z0loc'][g].rearrange("(a b) -> a b", a=12), zrow[:])

    if skip_head:
        return

    # ---- head: one AllGather, then replicated fp16 lin stack ----
    # prefetch the first plinw.bufs W chunks during the graph stack; later
    # chunks are issued after z0s so their buffer-waits can't head-of-line
    # block the collective path on the DMA queue.
    NCHUNK = LIN_D * (12 // LIN_CHUNK)

    def lin_dma(idx):
        li, half = divmod(idx, 12 // LIN_CHUNK)
        wt = plinw.tile([P, LIN_CHUNK * DIM2], F16, tag="linw", name="linw")
        dma(wt[:].rearrange("p (k m) -> p k m", k=LIN_CHUNK),
            io['linW'][li, LIN_CHUNK * half:LIN_CHUNK * (half + 1)]
            .rearrange("k p m -> p k m"))
        return wt

    lin_chunks = [lin_dma(i) for i in range(2)]
    cores = list(range(NCORES))
    nc.gpsimd.collective_compute(
        "AllGather", ALU.bypass, replica_groups=[cores],
        ins=[io['z0loc'][:]], outs=[io['z0all'][:]])
    z0s = pst.tile([32, DIM2], F32, tag="z0s", name="z0s", padded_shape=[128, DIM2])
    dma(z0s[:], io['z0all'][:])
    for i in range(2, NCHUNK):
        li_, half_ = divmod(i, 12 // LIN_CHUNK)
        wt_ = plinw.tile([P, LIN_CHUNK * DIM2], F16, tag="linw", name="linw")
        nc.gpsimd.dma_start(
            wt_[:].rearrange("p (k m) -> p k m", k=LIN_CHUNK),
            io['linW'][li_, LIN_CHUNK * half_:LIN_CHUNK * (half_ + 1)]
            .rearrange("k p m -> p k m"))
        lin_chunks.append(wt_)
    if 'z0dump' in io:
        dma(io['z0dump'][:], z0s[:])

    zT = []
    spack = pst.tile([P, 12], F32, tag="spack", name="spack")
    qpack = pst.tile([P, 12], F32, tag="qpack", name="qpack")
    for t in range(12):
        zt_ps = seq_ps(P, 32)
        nc.tensor.transpose(zt_ps[0:P, 0:32], z0s[:, 128 * t:128 * (t + 1)],
                            eye[0:32, 0:32])
        zt = pst.tile([P, 32], F32, tag=f"zT{t}", name=f"zT{t}")
        copy_ps(zt[:], zt_ps[0:P, 0:32])
        zT.append(zt)
        nc.vector.tensor_reduce(spack[:, t:t + 1], zt[:], axis=AXX, op=ALU.add)
        nc.scalar.activation(junk_a[:, 0:32], zt[:], ACTF.Square,
                             accum_out=qpack[:, t:t + 1])
    mu = pst.tile([P, 12], F32, tag="mu", name="mu")
    nc.vector.tensor_scalar(mu[:], spack[:], 1.0 / 32, 0.0, op0=ALU.mult)
    m2 = pst.tile([P, 12], F32, tag="m2", name="m2")
    nc.vector.tensor_tensor(m2[:], mu[:], mu[:], op=ALU.mult)
    var = pst.tile([P, 12], F32, tag="var", name="var")
    nc.vector.scalar_tensor_tensor(var[:], qpack[:], 1.0 / 32, m2[:],
                                   op0=ALU.mult, op1=ALU.subtract)
    eps_col = pst.tile([P, 1], F32, tag="eps_col", name="eps_col")
    nc.any.memset(eps_col[:], 1e-5)
    sd = pst.tile([P, 12], F32, tag="sd", name="sd")
    nc.scalar.activation(sd[:], var[:], ACTF.Sqrt, bias=eps_col[:])
    inv = pst.tile([P, 12], F32, tag="inv", name="inv")
    nc.vector.reciprocal(inv[:], sd[:])
    gam = wslice('bn_scale')
    bet = wslice('bn_shift')
    sc = pst.tile([P, 12], F32, tag="sc", name="sc")
    nc.vector.tensor_tensor(sc[:], inv[:], gam, op=ALU.mult)
    bi = pst.tile([P, 12], F32, tag="bi", name="bi")
    nc.vector.tensor_tensor(bi[:], mu[:], sc[:], op=ALU.mult)
    nc.vector.tensor_tensor(bi[:], bet, bi[:], op=ALU.subtract)
    zcur = []
    for t in range(12):
        zc = pst.tile([P, 32], F16, tag=f"zc{t}", name=f"zc{t}")
        nc.scalar.activation(zc[:], zT[t][:], ACTF.Identity,
                             bias=bi[:, t:t + 1], scale=sc[:, t:t + 1])
        zcur.append(zc)
    if 'hdump' in io:
        for t in range(12):
            dma(io['hdump'][0, t], zcur[t][:])

    linB = wslice('linB')
    for li in range(LIN_D):
        # z-stationary: out[b, m] in three [32, 512] psum banks (one group
        # per bank), weights are the moving operand streamed in k-chunks.
        pz = [psh.tile([32, N], F32, tag=f"pz{s}", name=f"pz{s}")
              for s in range(3)]
        for half in range(12 // LIN_CHUNK):
            wt = lin_chunks[li * (12 // LIN_CHUNK) + half]
            for kk in range(LIN_CHUNK):
                kt = LIN_CHUNK * half + kk
                wsl = wt[:, DIM2 * kk:DIM2 * (kk + 1)]
                for s in range(3):
                    nc.tensor.matmul(
                        pz[s][:], zcur[kt][:], wsl[:, 512 * s:512 * (s + 1)],
                        start=(kt == 0), stop=(kt == 11))
        zrow = pst.tile([32, DIM2], F32, tag="zrowh",
                        name="zrowh", padded_shape=[128, DIM2])
        for s in range(3):
            copy_ps(zrow[:, 512 * s:512 * (s + 1)], pz[s][:])
        znew = []
        for mt in range(12):
            ztp = seq_ps(P, 32)
            nc.tensor.transpose(ztp[0:P, 0:32],
                                zrow[0:32, 128 * mt:128 * (mt + 1)],
                                eye[0:32, 0:32])
            bcol = linB[:, 12 * li + mt:12 * li + mt + 1]
            zb = pst.tile([P, 32], F32, tag=f"zb{li % 2}_{mt}",
                          name=f"zb{li % 2}_{mt}")
            nc.scalar.activation(zb[:], ztp[0:P, 0:32], ACTF.Identity, bias=bcol)
            zc = pst.tile([P, 32], F16, tag=f"zn{li % 2}_{mt}",
                          name=f"zn{li % 2}_{mt}")
            nc.vector.scalar_tensor_tensor(zc[:], zb[:], 0.01, zb[:], **lrelu_op)
            znew.append(zc)
        if 'hdump' in io and li == 0:
            for t in range(12):
                dma(io['hdump'][1, t], znew[t][:])
        zcur = znew

    outW16 = pst.tile([P, 12], F16, tag="outW16", name="outW16")
    dma(outW16[:], io['outW16'][:])
    outb_sb = wslice('outb', 1)
    ps_out = seq_ps(1, 32)
    for k in range(12):
        nc.tensor.matmul(ps_out[:], outW16[:, k:k + 1], zcur[k][:],
                         start=(k == 0), stop=(k == 11))
    o32 = pwork.tile([1, 32], F32, tag="o32", name="o32", padded_shape=[128, 32])
    nc.scalar.activation(o32[:], ps_out[:], ACTF.Identity, bias=outb_sb)
    dma(io['out32'][:], o32[:])


def build_nc(skip_head=False):
    nc = bacc.Bacc("TRN2", target_bir_lowering=False, debug=False,
                   num_devices=NCORES)
    io = {}

    io['xpack'] = nc.dram_tensor('xpack', [GPC, 128, 2 * N + 24], F32,
                                 kind="ExternalInput").ap()
    io['wpack'] = nc.dram_tensor('wpack', [128, WPACK_COLS], F32,
                                 kind="ExternalInput").ap()
    io['linW'] = nc.dram_tensor('linW', [LIN_D, 12, 128, DIM2], F16,
                                kind="ExternalInput").ap()
    io['outW16'] = nc.dram_tensor('outW16', [128, 12], F16,
                                  kind="ExternalInput").ap()

    io['z0loc'] = nc.dram_tensor(
        "z0loc", [GPC, DIM2], F32,
        kind="ExternalOutput" if skip_head else "Internal").ap()
    io['out32'] = nc.dram_tensor("out32", [1, 32], F32,
                                 kind="ExternalOutput").ap()
    if not skip_head:
        io['z0all'] = nc.dram_tensor("z0all", [B, DIM2], F32,
                                     addr_space="Shared").ap()

    with tile.TileContext(nc) as tc:
        core_program(tc, io, skip_head=skip_head)
    nc.compile()
    return nc


_CACHED = {}
_DEBUG_IO = None


def kernel(**inputs) -> np.ndarray:
    from concourse.bass_utils import run_bass_kernel_spmd
    if 'nc' not in _CACHED:
        _CACHED['nc'] = build_nc()
    nc = _CACHED['nc']
    in_maps = []
    for c in range(NCORES):
        d = prep_host(inputs, c)
        in_maps.append({k: np.ascontiguousarray(v) for k, v in d.items()})
    res = run_bass_kernel_spmd(nc, in_maps, core_ids=list(range(NCORES)),
                               trace=bool(os.environ.get("KBENCH_TRACE")))
    _CACHED['last'] = res
    return res.results[0]['out32'].reshape(-1).astype(np.float32)


if __name__ == "__main__":
    data = dict(np.load('/root/problem/inputs.npz'))
    out = kernel(**data)
    print("kernel out:", out[:5])
